# revision 33
# baseline (speedup 1.0000x reference)
"""Trainium2 Bass kernel for nn_CausalFMMAttention.

Reference computation (per batch n, head h — all (n,h) pairs independent):
  phi1(x) = elu(x)+1 ; phi2(x) = (elu(x)+1)^2
  Two causal linear-attention branches (feature maps phi1 / phi2, K row-normalized,
  Q normalization cancels, key_lengths cancels under K-normalization, eps negligible):
      LVb[l] = (sum_{s<=l} (Qb_l . Kbn_s) V_s) / (Qb_l . cumsum(Kbn)_l)
  plus a width-10 banded causal softmax branch:
      SV[l]  = softmax_band(Q_l . K_s / sqrt(E)) @ V
  out = W1*SV + W2*LV1 + W3*LV2

Sharding: 16 (n,h) units, 2 per core across 8 cores (data-parallel N x
tensor-parallel H). Each core runs an identical program on its own 2 units.

Implementation: chunked scan over L in chunks of 128.
  - per chunk, PE computes A^T[s,l] = K.Q for the three branches via row-tiled
    (tile_position) matmuls on transposed operands; transposed operands are
    produced on-chip with col-tiled PE transpose-matmuls.
  - causal/band masking is fused into the (mandatory) PSUM->SBUF evacuations.
  - intra-chunk A@[V|1] and inter-chunk Q@[S|Kcum] accumulate into one PSUM
    tile per 4-chunk group; a [E, D+1] running state S accumulates in PSUM
    across chunks (K^T @ [V|1] matmuls).
  - the band crosses chunk boundaries by <=9 keys: handled with a tiny extra
    matmul against the previous chunk's K-tail / V-tail.
"""

import os
import sys
from contextlib import ExitStack

import numpy as np

if "/opt/trn_rl_repo" not in sys.path:
    sys.path.insert(0, "/opt/trn_rl_repo")

import concourse.bacc as bacc
import concourse.bass as bass
import concourse.mybir as mybir
import concourse.tile as tile
from concourse.bass_utils import run_bass_kernel_spmd
from concourse.masks import make_identity

F32 = mybir.dt.float32
ALU = mybir.AluOpType
AF = mybir.ActivationFunctionType

N, L, H, E = 2, 2048, 8, 32
D = E
NCORES = 8
UPC = (N * H) // NCORES  # units per core = 2
C = 128                  # chunk length
NCH = L // C             # 16 chunks
BW = 10                  # band width
TB = BW - 1              # boundary tail size = 9
TEMP = 1.0 / np.sqrt(np.float32(E))


def _units_of_core(c):
    return [((c * UPC + i) // H, (c * UPC + i) % H) for i in range(UPC)]


# ---------------------------------------------------------------------------
# kernel body (one core: UPC units)
# ---------------------------------------------------------------------------

def _build_unit(ctx, tc, pools, consts, q_ap, k_ap, v_ap, w_ap, o_ap, tag):
    nc = tc.nc
    ident, maskA, maskB, ones_row = consts
    (fpool, spool, qkt_pool, a_pool, araw_pool, p_pool, s_psum_pool,
     sb2_pool) = pools

    # ---------------- persistent per-unit tensors ----------------
    # qpack/kpack chunk layout (96 cols per chunk): [phi1 | phi2 | raw]
    qpack = fpool.tile([128, NCH * 96], F32, tag=f"qpack{tag}")
    kpack = fpool.tile([128, NCH * 96], F32, tag=f"kpack{tag}")
    vpu = fpool.tile([128, NCH * (D + 1)], F32, tag=f"vpu{tag}")
    vpsm = fpool.tile([128, NCH * (D + 1)], F32, tag=f"vpsm{tag}")
    qkt_all = fpool.tile([96, NCH * 256], F32, tag=f"qkt{tag}")
    out_sb = fpool.tile([128, NCH * E], F32, tag=f"out{tag}")
    wb = fpool.tile([128, 96], F32, tag=f"wb{tag}")

    def c3(t, x=E):  # [128, NCH*x] -> [128, NCH, x]
        return t[:].rearrange("p (c x) -> p c x", x=x)

    qv = c3(qpack, 96)
    kv = c3(kpack, 96)
    q1r, q2r, qrr = qv[:, :, 0:32], qv[:, :, 32:64], qv[:, :, 64:96]
    k1r, k2r, krr = kv[:, :, 0:32], kv[:, :, 32:64], kv[:, :, 64:96]

    # ---------------- loads ----------------
    qd = q_ap.rearrange("(c p) e -> p c e", p=128)
    kd = k_ap.rearrange("(c p) e -> p c e", p=128)
    vd = v_ap.rearrange("(c p) e -> p c e", p=128)
    nc.sync.dma_start(out=qrr, in_=qd)
    nc.sync.dma_start(out=krr, in_=kd)
    nc.sync.dma_start(out=c3(vpu, D + 1)[:, :, 0:D], in_=vd)
    nc.gpsimd.memset(c3(vpu, D + 1)[:, :, D : D + 1], 1.0)

    # W rows: w_ap is [3, E] (W1, W2, W3 for this unit's head)
    wrow = fpool.tile([1, 96], F32, tag=f"wrow{tag}")
    nc.sync.dma_start(out=wrow[0:1, :], in_=w_ap.rearrange("a e -> (a e)")[None, :])
    wb_ps = qkt_pool.tile([128, 96], F32, tag="qkt_ps")
    nc.tensor.matmul(wb_ps[:, :], lhsT=ones_row[0:1, 0:128], rhs=wrow[0:1, :],
                     start=True, stop=True)
    nc.scalar.copy(wb[:, :], wb_ps[:, :])

    # ---------------- feature maps (natural layout, whole unit) ----------------
    sc1 = spool.tile([128, NCH * E], F32, tag="sc1")
    # phi1(q) = exp(min(q,0)) + relu(q);  phi2 = phi1^2
    nc.scalar.activation(c3(sc1), qrr, AF.Exp)
    nc.vector.tensor_scalar_min(sc1[:, :], sc1[:, :], 1.0)
    nc.scalar.activation(q1r, qrr, AF.Relu)
    nc.vector.tensor_add(q1r, q1r, c3(sc1))
    nc.scalar.square(q2r, q1r)
    # phi1(k), phi2(k), then row-normalize (over E) each
    nc.scalar.activation(c3(sc1), krr, AF.Exp)
    nc.vector.tensor_scalar_min(sc1[:, :], sc1[:, :], 1.0)
    nc.scalar.activation(k1r, krr, AF.Relu)
    nc.vector.tensor_add(k1r, k1r, c3(sc1))
    nc.scalar.square(k2r, k1r)
    ssum = spool.tile([128, 2 * NCH], F32, tag="ssum")
    nc.vector.tensor_reduce(ssum[:, 0:NCH], k1r, axis=mybir.AxisListType.X,
                            op=ALU.add)
    nc.vector.tensor_reduce(ssum[:, NCH : 2 * NCH], k2r,
                            axis=mybir.AxisListType.X, op=ALU.add)
    srec = spool.tile([128, 2 * NCH], F32, tag="srec")
    nc.vector.reciprocal(srec[:, :], ssum[:, :])
    r1b = srec[:, None, 0:NCH].rearrange("p a c -> p c a").broadcast_to([128, NCH, E])
    r2b = (srec[:, None, NCH : 2 * NCH].rearrange("p a c -> p c a")
           .broadcast_to([128, NCH, E]))
    nc.vector.tensor_mul(k1r, k1r, r1b)
    nc.gpsimd.tensor_mul(k2r, k2r, r2b)

    # vpsm = V * W1 (softmax branch carries its W fold; ones col for denom)
    w1b = wb[:, None, 0:E].broadcast_to([128, NCH, E])
    nc.vector.tensor_mul(c3(vpsm, D + 1)[:, :, 0:D], c3(vpu, D + 1)[:, :, 0:D], w1b)
    nc.gpsimd.memset(c3(vpsm, D + 1)[:, :, D : D + 1], 1.0)

    # ---------------- chunk scan ----------------
    stage = int(os.environ.get("KERNEL_STAGE", "4"))
    if stage <= 1:
        nc.vector.tensor_copy(c3(out_sb), q1r)
        od = o_ap.rearrange("(c p) e -> p c e", p=128)
        nc.sync.dma_start(out=od, in_=c3(out_sb))
        return
    p_ps = None
    s_sb_prev = None
    nch_cap = int(os.environ.get("KERNEL_NCH", str(NCH)))
    for c in range(nch_cap):
        j = c % 4
        p0 = 96 * c

        # --- transposes: one [128,96]->[96,128] matmul per side ---
        qkt_ps = qkt_pool.tile([96, 256], F32, tag="qkt_ps")
        nc.tensor.transpose(qkt_ps[:, 0:128], qpack[:, p0 : p0 + 96], ident[:, :])
        nc.tensor.transpose(qkt_ps[:, 128:256], kpack[:, p0 : p0 + 96], ident[:, :])
        nc.scalar.copy(qkt_all[:, 256 * c : 256 * (c + 1)], qkt_ps[:, :])

        qt = qkt_all[:, 256 * c : 256 * c + 128]
        kt = qkt_all[:, 256 * c + 128 : 256 * (c + 1)]
        ktprev = qkt_all[:, 256 * (c - 1) + 128 : 256 * c]  # prev chunk KT

        sub = int(os.environ.get("KERNEL_SUB", "9"))
        if stage <= 2 and sub <= 1:
            nc.vector.tensor_copy(c3(out_sb)[0:96, c, :],
                                  qkt_all[:, 256 * c : 256 * c + 32])
            continue

        # --- A matmuls: A_b^T[s, l] = sum_e KbT[e,s] * QbT[e,l] ---
        # A1/A2/Araw run concurrently in different PE row groups, so each must
        # land in its own PSUM bank.
        a12_ps = a_pool.tile([128, 1024], F32, tag="a12_ps")
        araw_ps = araw_pool.tile([128, 512], F32, tag="araw_ps")
        for bi in range(2):
            b0 = 32 * bi
            nc.tensor.matmul(a12_ps[:, 512 * bi : 512 * bi + 128],
                             lhsT=kt[b0 : b0 + 32, :], rhs=qt[b0 : b0 + 32, :],
                             start=True, stop=True)
        nc.tensor.matmul(araw_ps[:, 0:128],
                         lhsT=kt[64:96, :], rhs=qt[64:96, :],
                         start=True, stop=True)
        if stage <= 2 and sub <= 11:
            nc.vector.tensor_copy(c3(out_sb)[:, c, :], a12_ps[:, 0:32])
            continue
        if c > 0:
            # boundary: all prev-chunk keys x first TB queries (band mask will
            # keep only the <=TB tail keys); full-height to init all of PSUM.
            # Same row group as the Araw matmul, so same bank is safe.
            nc.tensor.matmul(araw_ps[:, 128 : 128 + TB],
                             lhsT=ktprev[64:96, :],
                             rhs=qt[64:96, 0:TB], start=True, stop=True)
        if stage <= 2 and sub <= 12:
            nc.vector.tensor_copy(c3(out_sb)[:, c, :], a12_ps[:, 0:32])
            continue

        # --- masked evacuations ---
        a12m = sb2_pool.tile([128, 256], F32, tag="a12m")
        a12v = a12_ps[:].rearrange("p (b x) -> p b x", b=2)[:, :, 0:128]
        nc.vector.tensor_mul(a12m[:].rearrange("p (b x) -> p b x", b=2),
                             a12v, maskA[:].rearrange("p (b x) -> p b x", b=2))
        if stage <= 2 and sub <= 2:
            nc.vector.tensor_copy(c3(out_sb)[:, c, :], a12m[:, 0:32])
            continue
        eband = sb2_pool.tile([128, 128 + TB], F32, tag="eband")
        nwid = 128 + TB if c > 0 else 128
        nc.scalar.activation(eband[:, 0:nwid], araw_ps[:, 0:nwid], AF.Exp,
                             scale=float(TEMP))
        nc.gpsimd.tensor_mul(eband[:, 0:nwid], eband[:, 0:nwid], maskB[:, 0:nwid])

        if stage <= 2:
            nc.vector.tensor_copy(c3(out_sb)[:, c, :], a12m[:, 0:32])
            continue

        s_sb = s_sb_prev  # state after chunks < c (None for c == 0)

        # --- P matmuls into the group PSUM tile ---
        # One PSUM accumulation group spans the whole p_ps tile (4 chunks x 3
        # branches): start on the first matmul, stop on the last full-height
        # one; everything in between accumulates (fresh columns are
        # "pending-zero" so the first write to them overwrites).
        if j == 0:
            p_ps = p_pool.tile([128, 4 * 3 * (D + 1)], F32, tag="p_ps")
        pc0 = 3 * (D + 1) * j

        pcol = pc0 + (D + 1) * 2
        nc.tensor.matmul(p_ps[:, pcol : pcol + D + 1], lhsT=eband[:, 0:128],
                         rhs=vpsm[:, (D + 1) * c : (D + 1) * (c + 1)],
                         start=(j == 0), stop=False)
        if c > 0:
            nc.tensor.matmul(p_ps[0:TB, pcol : pcol + D + 1],
                             lhsT=eband[:, 128 : 128 + TB],
                             rhs=vpsm[:, (D + 1) * (c - 1) : (D + 1) * c],
                             start=False, stop=False)
        for bi in range(2):
            pcol = pc0 + (D + 1) * bi
            nc.tensor.matmul(p_ps[:, pcol : pcol + D + 1],
                             lhsT=a12m[:, 128 * bi : 128 * (bi + 1)],
                             rhs=vpu[:, (D + 1) * c : (D + 1) * (c + 1)],
                             start=False, stop=False)
            if s_sb is not None:
                b0 = 32 * bi
                nc.tensor.matmul(p_ps[:, pcol : pcol + D + 1],
                                 lhsT=qt[b0 : b0 + 32, :],
                                 rhs=s_sb[b0 : b0 + 32, :],
                                 start=False,
                                 stop=(j == 3 and bi == 1))

        # --- state update: [S1; S2] += [K1n | K2n]^T @ [V | 1], one matmul ---
        if c < NCH - 1:
            supd_ps = s_psum_pool.tile([64, D + 1], F32, tag="supd_ps")
            nc.tensor.matmul(supd_ps[:, :], lhsT=kpack[:, p0 : p0 + 64],
                             rhs=vpu[:, (D + 1) * c : (D + 1) * (c + 1)],
                             start=True, stop=True)
            s_sb_new = sb2_pool.tile([64, D + 1], F32, tag="s_sb")
            if c == 0:
                nc.vector.tensor_copy(s_sb_new[:, :], supd_ps[:, :])
            else:
                nc.vector.tensor_add(s_sb_new[:, :], s_sb_prev[:, :], supd_ps[:, :])
            s_sb_prev = s_sb_new

        # --- per-group epilogue: z = 1/den, out = sum_b W_b*num_b*z_b ---
        if j == 3:
            g = c // 4
            p4 = p_ps[:].rearrange("p (j b x) -> p j b x", j=4, x=D + 1)
            z12 = sb2_pool.tile([128, 12], F32, tag="z12")
            z4 = z12[:].rearrange("p (j b) -> p j b", j=4)
            nc.vector.reciprocal(z4[:, :, :, None], p4[:, :, :, D : D + 1])
            obig = sb2_pool.tile([128, 4 * 3 * D], F32, tag="obig")
            o4 = obig[:].rearrange("p (j b x) -> p j b x", j=4, x=D)
            nc.vector.tensor_mul(o4, p4[:, :, :, 0:D],
                                 z4[:, :, :, None].broadcast_to([128, 4, 3, D]))
            # W2/W3 scaling for the two linear branches (softmax already has W1)
            w23 = (wb[:, None, None, E : 3 * E]
                   .rearrange("p a b (w x) -> p a (b w) x", x=D)
                   .broadcast_to([128, 4, 2, D]))
            nc.vector.tensor_mul(o4[:, :, 0:2, :], o4[:, :, 0:2, :], w23)
            t1 = sb2_pool.tile([128, 4 * D], F32, tag="t1")
            t13 = t1[:].rearrange("p (j x) -> p j x", x=D)
            nc.gpsimd.tensor_add(t13, o4[:, :, 0, :], o4[:, :, 1, :])
            nc.gpsimd.tensor_add(c3(out_sb)[:, 4 * g : 4 * (g + 1), :], t13,
                                 o4[:, :, 2, :])

    # ---------------- store ----------------
    od = o_ap.rearrange("(c p) e -> p c e", p=128)
    nc.sync.dma_start(out=od, in_=c3(out_sb))


def build_core_kernel(ctx, tc, outs, ins):
    """outs/ins: dicts of DRAM APs. ins: q, k, v [UPC, L, E]; w [UPC, 3, E]."""
    nc = tc.nc
    const_pool = ctx.enter_context(tc.tile_pool(name="const", bufs=1))
    fpool = ctx.enter_context(tc.tile_pool(name="fpers", bufs=1))
    spool = ctx.enter_context(tc.tile_pool(name="fscratch", bufs=2))
    qkt_pool = ctx.enter_context(tc.tile_pool(name="qkt", bufs=2, space="PSUM"))
    # A1/A2 go in two different PSUM banks of one [128, 1024] tile and Araw in
    # a third bank: concurrent matmuls in different PE row groups writing the
    # same PSUM bank hard-fault the device.
    a_pool = ctx.enter_context(tc.tile_pool(name="aps", bufs=1, space="PSUM"))
    araw_pool = ctx.enter_context(tc.tile_pool(name="araw", bufs=1, space="PSUM"))
    p_pool = ctx.enter_context(tc.tile_pool(name="pps", bufs=2, space="PSUM"))
    s_psum_pool = ctx.enter_context(tc.tile_pool(name="spsum", bufs=1, space="PSUM"))
    sb2_pool = ctx.enter_context(tc.tile_pool(name="sb2", bufs=3))

    ident = const_pool.tile([128, 128], F32, tag="ident")
    make_identity(nc, ident[:, :])
    ones_row = const_pool.tile([1, 128], F32, tag="ones_row")
    nc.gpsimd.memset(ones_row[:, :], 1.0)

    # causal keep-mask (s <= l), duplicated along cols for both branches
    maskA = const_pool.tile([128, 256], F32, tag="maskA")
    nc.gpsimd.memset(maskA[:, :], 1.0)
    nc.gpsimd.affine_select(
        out=maskA[:, :], in_=maskA[:, :], compare_op=ALU.is_ge, fill=0.0,
        base=0, pattern=[[0, 2], [1, 128]], channel_multiplier=-1)

    # band mask: cols 0..127: 1 where 0 <= l-s <= BW-1 ; cols 128..136:
    # boundary block, keep j >= l on partitions 0..TB-1, zero elsewhere.
    maskB = const_pool.tile([128, 128 + TB], F32, tag="maskB")
    nc.gpsimd.memset(maskB[:, :], 0.0)
    nc.gpsimd.memset(maskB[:, 0:128], 1.0)
    nc.gpsimd.affine_select(
        out=maskB[:, 0:128], in_=maskB[:, 0:128], compare_op=ALU.is_ge,
        fill=0.0, base=0, pattern=[[1, 128]], channel_multiplier=-1)
    nc.gpsimd.affine_select(
        out=maskB[:, 0:128], in_=maskB[:, 0:128], compare_op=ALU.is_ge,
        fill=0.0, base=BW - 1, pattern=[[-1, 128]], channel_multiplier=1)
    # boundary block: keep prev-chunk key s_prev (= partition p) for query
    # l (= col) iff p >= (C - TB) + l  <=>  p - (C - TB) - l >= 0
    nc.gpsimd.memset(maskB[:, 128 : 128 + TB], 1.0)
    nc.gpsimd.affine_select(
        out=maskB[:, 128 : 128 + TB], in_=maskB[:, 128 : 128 + TB],
        compare_op=ALU.is_ge, fill=0.0, base=-(C - TB), pattern=[[-1, TB]],
        channel_multiplier=1)

    consts = (ident, maskA, maskB, ones_row)
    pools = (fpool, spool, qkt_pool, a_pool, araw_pool, p_pool, s_psum_pool,
             sb2_pool)
    for u in range(int(os.environ.get("KERNEL_UNITS", str(UPC)))):
        _build_unit(ctx, tc, pools, consts,
                    ins["q"][u], ins["k"][u], ins["v"][u], ins["w"][u],
                    outs["o"][u], tag=u)


# ---------------------------------------------------------------------------
# host-side entry point
# ---------------------------------------------------------------------------

_CACHE = {}


def _get_nc():
    if "nc" in _CACHE:
        return _CACHE["nc"]
    nc = bacc.Bacc("TRN2", target_bir_lowering=False, debug=False,
                   enable_asserts=True, num_devices=NCORES)
    ins = {
        "q": nc.dram_tensor("q", [UPC, L, E], F32, kind="ExternalInput").ap(),
        "k": nc.dram_tensor("k", [UPC, L, E], F32, kind="ExternalInput").ap(),
        "v": nc.dram_tensor("v", [UPC, L, E], F32, kind="ExternalInput").ap(),
        "w": nc.dram_tensor("w", [UPC, 3, E], F32, kind="ExternalInput").ap(),
    }
    outs = {"o": nc.dram_tensor("o", [UPC, L, E], F32, kind="ExternalOutput").ap()}
    with tile.TileContext(nc) as tc:
        with ExitStack() as ctx:
            build_core_kernel(ctx, tc, outs, ins)
    nc.compile()
    _CACHE["nc"] = nc
    return nc


def make_in_maps(queries, keys, values, W1, W2, W3):
    in_maps = []
    for core in range(NCORES):
        units = _units_of_core(core)
        in_maps.append({
            "q": np.stack([queries[n, :, h, :] for (n, h) in units]).copy(),
            "k": np.stack([keys[n, :, h, :] for (n, h) in units]).copy(),
            "v": np.stack([values[n, :, h, :] for (n, h) in units]).copy(),
            "w": np.stack([
                np.stack([W1[0, 0, h], W2[0, 0, h], W3[0, 0, h]])
                for (n, h) in units]).copy(),
        })
    return in_maps


def kernel(**inputs):
    queries = np.asarray(inputs["queries"], dtype=np.float32)
    keys = np.asarray(inputs["keys"], dtype=np.float32)
    values = np.asarray(inputs["values"], dtype=np.float32)
    W1 = np.asarray(inputs["W1"], dtype=np.float32)
    W2 = np.asarray(inputs["W2"], dtype=np.float32)
    W3 = np.asarray(inputs["W3"], dtype=np.float32)

    nc = _get_nc()
    in_maps = make_in_maps(queries, keys, values, W1, W2, W3)
    res = run_bass_kernel_spmd(nc, in_maps, core_ids=list(range(NCORES)),
                               trace=bool(int(os.environ.get("KERNEL_TRACE", "0"))))
    _CACHE["last_results"] = res
    out = np.zeros((N, L, H, E), dtype=np.float32)
    for core in range(NCORES):
        r = res.results[core]["o"]
        for i, (n, h) in enumerate(_units_of_core(core)):
            out[n, :, h, :] = r[i]
    return out
